# revision 3
# baseline (speedup 1.0000x reference)
"""Trainium2 Bass kernel for nn_AdaptiveBlock (8 NeuronCores, SPMD).

Sharding: data-parallel over batch (2 batches/core) for the attention
front-end; AllGather of the blended activations A^T; vocab-column-parallel
final classifier (6400 vocab rows/core of the padded 51200).
"""

import sys
import types

import numpy as np

sys.path.insert(0, "/opt/trn_rl_repo")

import ml_dtypes  # noqa: E402

bf16 = ml_dtypes.bfloat16

# ---- problem constants (hardcoded per task rules) ----
B, TD, TT, H, E2, K, VOC = 16, 128, 32, 1024, 1024, 49, 50257
NC_ = 8                   # cores
BPC = B // NC_            # batches per core = 2
TPB = TD + TT             # tokens per batch = 160
TPC = BPC * TPB           # tokens per core = 320
TOK = B * TPB             # total tokens = 2560
VPAD = 51200              # padded vocab (8 * 6400)
VPC = VPAD // NC_         # vocab per core = 6400
NH = H // 128             # h chunks = 8
NM = TOK // 128           # token chunks = 20
VCHUNKS = [512] * 12 + [256]   # 12*512 + 256 = 6400
KJ = K * K                # 2401
TJ = TT * K               # 1568


def _install_axon_trace_hook():
    if "antenv.axon_hooks" in sys.modules:
        return
    try:
        import antenv
        mod = types.ModuleType("antenv.axon_hooks")
        state = {"hook": None}
        mod.set_axon_ntff_profile_hook = lambda h: state.__setitem__("hook", h)
        mod.get_axon_ntff_profile_hook = lambda: state["hook"]
        sys.modules["antenv.axon_hooks"] = mod
        antenv.axon_hooks = mod
        if "/root/.axon_site" not in sys.path:
            sys.path.insert(0, "/root/.axon_site")
        from trn_agent_boot.trn_boot import _ntff_profile_via_ctypes
        mod.set_axon_ntff_profile_hook(
            _ntff_profile_via_ctypes("/opt/axon/libaxon_pjrt.so"))
    except Exception:
        pass


def _split_oversized_waits(nc, mybir, maxw=1):
    """This walrus build allows one sync wait per instruction; hoist extras
    onto preceding same-engine NOPs (engines execute their stream in order)."""
    n_new = 0
    for f in nc.m.functions:
        for bb in f.blocks:
            new_insts = []
            for inst in bb.instructions:
                si = inst.sync_info
                if si is not None and len(si.on_wait) > maxw:
                    waits = list(si.on_wait)
                    extra, keep = waits[:-maxw], waits[-maxw:]
                    while extra:
                        chunk, extra = extra[:maxw], extra[maxw:]
                        nop = mybir.InstNoOp(
                            name=f"{inst.name}-wsplit-{n_new}",
                            engine=inst.engine,
                            sync_info=mybir.SyncInfo(on_wait=chunk, on_update=[]),
                        )
                        n_new += 1
                        nc.register_instruction(nop)
                        new_insts.append(nop)
                    si.on_wait = keep
                new_insts.append(inst)
            bb.instructions = new_insts
    return n_new


_BUILT = None


def _build():
    """Build the Bass graph (one SPMD program, same on all 8 cores)."""
    global _BUILT
    if _BUILT is not None:
        return _BUILT

    import concourse.bass as bass
    import concourse.mybir as mybir
    from concourse.tile import TileContext
    from concourse.tile_rust import add_dep_helper
    from concourse.masks import make_identity

    F32 = mybir.dt.float32
    BF = mybir.dt.bfloat16
    AF = mybir.ActivationFunctionType
    ALU = mybir.AluOpType
    AX = mybir.AxisListType

    nc = bass.Bass()
    P = nc.declare_dram_parameter

    # ---------------- parameters (per-core shards, host-prepared) --------
    XT = P("XT", [H, TPC], BF, isOutput=False)        # x^T  (concat tokens)
    HPT = P("HPT", [H, TPC], BF, isOutput=False)      # h_prev^T
    HT = P("HT", [H, TPC], BF, isOutput=False)        # hiddens^T
    CT = P("CT", [H, TPC], F32, isOutput=False)       # cells^T
    VN = P("VN", [BPC, K, H], BF, isOutput=False)     # V natural
    VT = P("VT", [BPC, H, K], BF, isOutput=False)     # V^T
    HDN = P("HDN", [BPC, TD, H], BF, isOutput=False)  # hiddens_des natural
    WXT = P("WXT", [H, H], BF, isOutput=False)
    WHST = P("WHST", [H, H], BF, isOutput=False)
    WVDT = P("WVDT", [H, K], BF, isOutput=False)
    WGDT = P("WGDT", [H, K], BF, isOutput=False)
    WSDT = P("WSDT", [H, K], BF, isOutput=False)
    WVTT = P("WVTT", [H, K], BF, isOutput=False)
    WGTT = P("WGTT", [H, K], BF, isOutput=False)
    WHD49 = P("WHD49", [KJ], F32, isOutput=False)     # tile(Wh_d, 49)
    WHD32 = P("WHD32", [TJ], F32, isOutput=False)     # tile(Wh_d, 32)
    WHT32 = P("WHT32", [TJ], F32, isOutput=False)     # tile(Wh_t, 32)
    WTM = P("WTM", [H, VPC], BF, isOutput=False)      # Wmlp^T vocab slice
    BI = P("BI", [VPC], F32, isOutput=False)          # bias slice
    OUT = P("OUT", [TOK, VPC], F32, isOutput=True)
    ATDBG = P("ATDBG", [H, TPC], BF, isOutput=True)   # debug: local A^T

    # ---------------- internal DRAM ----------------
    vw_dr = nc.dram_tensor("vw_dr", [BPC, K, K], F32)        # Vw roundtrip
    hgt_dr = nc.dram_tensor("hgt_dr", [BPC, TT, K], F32)     # hg_tit flat
    htg_dr = nc.dram_tensor("htg_dr", [BPC, TT, K], F32)     # htg flat
    beta_dr = nc.dram_tensor("beta_dr", [BPC, 2, TD], F32)   # beta des/tit
    at_bounce = nc.dram_tensor("at_bounce", [H, TPC], BF)
    ag_out = nc.dram_tensor("ag_out", [NC_, H, TPC], BF, addr_space="Shared")

    with TileContext(nc) as tc:
        with tc.tile_pool(name="const", bufs=1) as cp, \
             tc.tile_pool(name="scr", bufs=1) as scr:

            dma = nc.sync.dma_start

            def tl(pool, shape, dt, tag, bufs=1):
                return pool.tile(shape, dt, tag=tag, name=tag, bufs=bufs)

            # ---- long-lived constants ----
            ident = tl(cp, [128, 128], F32, "ident")
            make_identity(nc, ident[:])
            whd49 = tl(cp, [128, KJ], F32, "whd49")
            dma(out=whd49[:], in_=WHD49[:].partition_broadcast(128))
            whd32 = tl(cp, [K, TJ], F32, "whd32")
            dma(out=whd32[:], in_=WHD32[:].partition_broadcast(K))
            wht32 = tl(cp, [128, TJ], F32, "wht32")
            dma(out=wht32[:], in_=WHT32[:].partition_broadcast(128))

            def load_chunks(pool, pref, src, width, dt=BF):
                ts_ = []
                for h in range(NH):
                    t = tl(pool, [128, width], dt, f"{pref}{h}")
                    dma(out=t[:], in_=src[h*128:(h+1)*128, :])
                    ts_.append(t)
                return ts_

            wvdt = load_chunks(cp, "wvdt", WVDT, K)
            wgdt = load_chunks(cp, "wgdt", WGDT, K)
            wsdt = load_chunks(cp, "wsdt", WSDT, K)
            wvtt = load_chunks(cp, "wvtt", WVTT, K)
            wgtt = load_chunks(cp, "wgtt", WGTT, K)
            ht = load_chunks(cp, "ht", HT, TPC)
            vsb, vtsb, hdsb = [], [], []
            for b in range(BPC):
                v = tl(cp, [K, H], BF, f"vn{b}")
                dma(out=v[:], in_=VN[b])
                vsb.append(v)
                vt = []
                for h in range(NH):
                    t = tl(cp, [128, K], BF, f"vt{b}_{h}")
                    dma(out=t[:], in_=VT[b, h*128:(h+1)*128, :])
                    vt.append(t)
                vtsb.append(vt)
                hd = tl(cp, [TD, H], BF, f"hd{b}")
                dma(out=hd[:], in_=HDN[b])
                hdsb.append(hd)

            # attention outputs/persistents (freed before classifier)
            with tc.tile_pool(name="attn", bufs=1) as ap_, \
                 tc.tile_pool(name="aps", bufs=1, space="PSUM") as aps:

                st = [tl(ap_, [128, TPC], BF, f"st{o}") for o in range(NH)]
                atout = [tl(ap_, [128, TPC], BF, f"ato{h}") for h in range(NH)]

                # ============ Phase A1: sentinel (own pool, freed early) ====
                with tc.tile_pool(name="sent", bufs=1) as sp:
                    wxt = load_chunks(sp, "wxt", WXT, H)
                    whst = load_chunks(sp, "whst", WHST, H)
                    xt = load_chunks(sp, "xt", XT, TPC)
                    hpt = load_chunks(sp, "hpt", HPT, TPC)
                    ct = load_chunks(sp, "ct", CT, TPC, F32)
                    for o in range(NH):
                        pg = tl(aps, [128, TPC], F32, "pgate", bufs=1)
                        for e in range(NH):
                            nc.tensor.matmul(pg[:], wxt[e][:, o*128:(o+1)*128],
                                             xt[e][:], start=(e == 0), stop=False)
                        for e in range(NH):
                            nc.tensor.matmul(pg[:], whst[e][:, o*128:(o+1)*128],
                                             hpt[e][:], start=False,
                                             stop=(e == NH - 1))
                        sig = tl(scr, [128, TPC], F32, "sig", bufs=2)
                        nc.scalar.activation(sig[:], pg[:], AF.Sigmoid)
                        tcl = tl(scr, [128, TPC], F32, "tcl", bufs=2)
                        nc.scalar.activation(tcl[:], ct[o][:], AF.Tanh)
                        nc.vector.tensor_tensor(out=st[o][:], in0=sig[:],
                                                in1=tcl[:], op=ALU.mult)

                # ============ Phase A2: attention per batch ============
                def mm_acc(ps, lhs_list, rhs_list):
                    n = len(lhs_list)
                    for e in range(n):
                        nc.tensor.matmul(ps, lhs_list[e], rhs_list[e],
                                         start=(e == 0), stop=(e == n - 1))

                def transpose(src_ap, p, f):
                    """src [p, f] f32 -> sbuf [f, p] f32"""
                    pt = tl(aps, [f, p], F32, "ptr", bufs=2)
                    nc.tensor.transpose(pt[:], src_ap, ident[:p, :p])
                    out = tl(scr, [f, p], F32, "str", bufs=2)
                    nc.vector.tensor_copy(out[:], pt[:])
                    return out

                for b in range(BPC):
                    c0 = b * TPB
                    c1 = c0 + TD
                    # --- small matmuls (shared psum tag group) ---
                    p_vw = tl(aps, [K, K], F32, "psm", bufs=3)
                    mm_acc(p_vw[:], [vtsb[b][e][:] for e in range(NH)],
                           [wvdt[e][:] for e in range(NH)])
                    vws = tl(ap_, [K, K], F32, "vws")
                    nc.vector.tensor_copy(vws[:], p_vw[:])
                    d1 = dma(out=vw_dr[b], in_=vws[:])
                    vwb = tl(ap_, [128, KJ], F32, "vwb")
                    d2 = dma(out=vwb[:], in_=vw_dr[b].rearrange("k j -> (k j)")
                             .partition_broadcast(128))
                    add_dep_helper(d2.ins, d1.ins, reason="vw roundtrip")

                    p_hgd = tl(aps, [TD, K], F32, "psm", bufs=3)
                    mm_acc(p_hgd[:], [ht[e][:, c0:c0+TD] for e in range(NH)],
                           [wgdt[e][:] for e in range(NH)])
                    hgd = tl(ap_, [TD, K], F32, "hgd")
                    nc.vector.tensor_copy(hgd[:], p_hgd[:])

                    p_hgt = tl(aps, [TT, K], F32, "psm", bufs=3)
                    mm_acc(p_hgt[:], [ht[e][:, c1:c1+TT] for e in range(NH)],
                           [wgdt[e][:] for e in range(NH)])
                    hgt = tl(ap_, [TT, K], F32, "hgt")
                    nc.vector.tensor_copy(hgt[:], p_hgt[:])
                    d3 = dma(out=hgt_dr[b], in_=hgt[:])
                    hgtb = tl(ap_, [K, TJ], F32, "hgtb")
                    d4 = dma(out=hgtb[:], in_=hgt_dr[b].rearrange("t j -> (t j)")
                             .partition_broadcast(K))
                    add_dep_helper(d4.ins, d3.ins, reason="hgt roundtrip")

                    p_htg = tl(aps, [TT, K], F32, "psm", bufs=3)
                    mm_acc(p_htg[:], [ht[e][:, c1:c1+TT] for e in range(NH)],
                           [wgtt[e][:] for e in range(NH)])
                    htgs = tl(ap_, [TT, K], F32, "htgs")
                    nc.vector.tensor_copy(htgs[:], p_htg[:])
                    d5 = dma(out=htg_dr[b], in_=htgs[:])
                    htgb = tl(ap_, [128, TJ], F32, "htgb")
                    d6 = dma(out=htgb[:], in_=htg_dr[b].rearrange("t j -> (t j)")
                             .partition_broadcast(128))
                    add_dep_helper(d6.ins, d5.ins, reason="htg roundtrip")

                    p_hdw = tl(aps, [TD, K], F32, "psm", bufs=3)
                    mm_acc(p_hdw[:], [ht[e][:, c0:c0+TD] for e in range(NH)],
                           [wvtt[e][:] for e in range(NH)])
                    hdw = tl(ap_, [TD, K], F32, "hdw")
                    nc.vector.tensor_copy(hdw[:], p_hdw[:])

                    p_sgd = tl(aps, [TD, K], F32, "psm", bufs=3)
                    mm_acc(p_sgd[:], [st[e][:, c0:c0+TD] for e in range(NH)],
                           [wsdt[e][:] for e in range(NH)])
                    sgd = tl(ap_, [TD, K], F32, "sgd")
                    nc.vector.tensor_copy(sgd[:], p_sgd[:])
                    p_sgt = tl(aps, [TT, K], F32, "psm", bufs=3)
                    mm_acc(p_sgt[:], [st[e][:, c1:c1+TT] for e in range(NH)],
                           [wsdt[e][:] for e in range(NH)])
                    sgt = tl(ap_, [TT, K], F32, "sgt")
                    nc.vector.tensor_copy(sgt[:], p_sgt[:])

                    # --- z des: [TD, K] over V ---
                    cv = tl(scr, [TD, KJ], F32, "cv")
                    nc.vector.tensor_tensor(
                        out=cv[:].rearrange("p (k j) -> p k j", k=K),
                        in0=vwb[:TD, :].rearrange("p (k j) -> p k j", k=K),
                        in1=hgd[:].unsqueeze(1).broadcast_to([TD, K, K]),
                        op=ALU.add)
                    nc.scalar.activation(cv[:], cv[:], AF.Tanh)
                    nc.vector.tensor_tensor(out=cv[:], in0=cv[:],
                                            in1=whd49[:TD, :], op=ALU.mult)
                    z_des = tl(ap_, [TD, K], F32, "z_des")
                    nc.vector.tensor_reduce(
                        out=z_des[:],
                        in_=cv[:].rearrange("p (k j) -> p k j", k=K),
                        op=ALU.add, axis=AX.X)

                    # --- z tit over V: k-partition layout [K, TT*K] ---
                    cvt = tl(scr, [K, TJ], F32, "cvt")
                    nc.vector.tensor_tensor(
                        out=cvt[:].rearrange("p (t j) -> p t j", t=TT),
                        in0=vws[:].unsqueeze(1).broadcast_to([K, TT, K]),
                        in1=hgtb[:].rearrange("p (t j) -> p t j", t=TT),
                        op=ALU.add)
                    nc.scalar.activation(cvt[:], cvt[:], AF.Tanh)
                    nc.vector.tensor_tensor(out=cvt[:], in0=cvt[:],
                                            in1=whd32[:, :], op=ALU.mult)
                    ztt = tl(scr, [K, TT], F32, "ztt")
                    nc.vector.tensor_reduce(
                        out=ztt[:],
                        in_=cvt[:].rearrange("p (t j) -> p t j", t=TT),
                        op=ALU.add, axis=AX.X)
                    z_tit = transpose(ztt[:], K, TT)             # [TT, K]

                    # --- z2 tit over hd: u-partition layout [TD, TT*K] ---
                    cv2 = tl(scr, [TD, TJ], F32, "cv2")
                    nc.vector.tensor_tensor(
                        out=cv2[:].rearrange("p (t j) -> p t j", t=TT),
                        in0=hdw[:].unsqueeze(1).broadcast_to([TD, TT, K]),
                        in1=htgb[:TD, :].rearrange("p (t j) -> p t j", t=TT),
                        op=ALU.add)
                    nc.scalar.activation(cv2[:], cv2[:], AF.Tanh)
                    nc.vector.tensor_tensor(out=cv2[:], in0=cv2[:],
                                            in1=wht32[:TD, :], op=ALU.mult)
                    z2t = tl(scr, [TD, TT], F32, "z2t")
                    nc.vector.tensor_reduce(
                        out=z2t[:],
                        in_=cv2[:].rearrange("p (t j) -> p t j", t=TT),
                        op=ALU.add, axis=AX.X)
                    z2 = transpose(z2t[:], TD, TT)               # [TT, TD]

                    # --- zs (sentinel scores) ---
                    def zs_path(sg_sb, hg_sb, t, tag):
                        cvs = tl(scr, [t, K], F32, f"cvs{tag}")
                        nc.vector.tensor_tensor(out=cvs[:], in0=sg_sb[:],
                                                in1=hg_sb[:], op=ALU.add)
                        nc.scalar.activation(cvs[:], cvs[:], AF.Tanh)
                        nc.vector.tensor_tensor(out=cvs[:], in0=cvs[:],
                                                in1=whd49[:t, :K], op=ALU.mult)
                        zs = tl(scr, [t, 1], F32, f"zs{tag}")
                        nc.vector.tensor_reduce(out=zs[:], in_=cvs[:],
                                                op=ALU.add, axis=AX.X)
                        return zs

                    zs_des = zs_path(sgd, hgd, TD, "d")
                    zs_tit = zs_path(sgt, hgt, TT, "t")

                    # --- softmax + beta, per branch ---
                    def soft_beta(z, zs, t, beta_slot, tag):
                        m1 = tl(scr, [t, 1], F32, f"m1{tag}")
                        nc.vector.tensor_reduce(out=m1[:], in_=z[:], op=ALU.max,
                                                axis=AX.X, negate=True)
                        e = tl(scr, [t, K], F32, f"esm{tag}")
                        s1 = tl(scr, [t, 1], F32, f"s1{tag}")
                        nc.scalar.activation(e[:], z[:], AF.Exp, bias=m1[:],
                                             accum_out=s1[:])
                        es = tl(scr, [t, 1], F32, f"es{tag}")
                        nc.scalar.activation(es[:], zs[:], AF.Exp, bias=m1[:])
                        den = tl(scr, [t, 1], F32, f"den{tag}")
                        nc.vector.tensor_tensor(out=den[:], in0=s1[:],
                                                in1=es[:], op=ALU.add)
                        rden = tl(scr, [t, 1], F32, f"rden{tag}")
                        nc.vector.reciprocal(rden[:], den[:])
                        beta = tl(scr, [t, 1], F32, f"beta{tag}")
                        nc.vector.tensor_tensor(out=beta[:], in0=es[:],
                                                in1=rden[:], op=ALU.mult)
                        rs1 = tl(scr, [t, 1], F32, f"rs1{tag}")
                        nc.vector.reciprocal(rs1[:], s1[:])
                        alpha = tl(scr, [t, K], F32, f"alpha{tag}")
                        nc.vector.tensor_scalar(out=alpha[:], in0=e[:],
                                                scalar1=rs1[:], scalar2=None,
                                                op0=ALU.mult)
                        alt_f = transpose(alpha[:], t, K)
                        alt = tl(scr, [K, t], BF, f"altb{tag}", bufs=2)
                        nc.vector.tensor_copy(alt[:], alt_f[:])
                        da = dma(out=beta_dr[b, beta_slot, :t], in_=beta[:])
                        bb_ = tl(scr, [128, t], F32, f"betab{tag}", bufs=2)
                        db = dma(out=bb_[:], in_=beta_dr[b, beta_slot, :t]
                                 .partition_broadcast(128))
                        add_dep_helper(db.ins, da.ins, reason="beta roundtrip")
                        return alt, bb_

                    altd, betabd = soft_beta(z_des, zs_des, TD, 0, "d")
                    altt, betabt = soft_beta(z_tit, zs_tit, TT, 1, "t")

                    # alpha2 = softmax(z2) [TT, TD] -> alpha2T [TD, TT] bf16
                    m2 = tl(scr, [TT, 1], F32, "m2")
                    nc.vector.tensor_reduce(out=m2[:], in_=z2[:], op=ALU.max,
                                            axis=AX.X, negate=True)
                    e2_ = tl(scr, [TT, TD], F32, "e2")
                    s2 = tl(scr, [TT, 1], F32, "s2")
                    nc.scalar.activation(e2_[:], z2[:], AF.Exp, bias=m2[:],
                                         accum_out=s2[:])
                    rs2 = tl(scr, [TT, 1], F32, "rs2")
                    nc.vector.reciprocal(rs2[:], s2[:])
                    al2 = tl(scr, [TT, TD], F32, "al2")
                    nc.vector.tensor_scalar(out=al2[:], in0=e2_[:],
                                            scalar1=rs2[:], scalar2=None,
                                            op0=ALU.mult)
                    al2t_f = transpose(al2[:], TT, TD)
                    al2t = tl(scr, [TD, TT], BF, "al2tb", bufs=2)
                    nc.vector.tensor_copy(al2t[:], al2t_f[:])

                    # --- contexts + blends per h-chunk ---
                    for h in range(NH):
                        hs = slice(h*128, (h+1)*128)
                        pcd = tl(aps, [128, TD], F32, "pctx", bufs=2)
                        nc.tensor.matmul(pcd[:], vsb[b][:, hs], altd[:],
                                         start=True, stop=True)
                        pct = tl(aps, [128, TT], F32, "pctx", bufs=2)
                        nc.tensor.matmul(pct[:], vsb[b][:, hs], altt[:],
                                         start=True, stop=True)
                        pc2 = tl(aps, [128, TT], F32, "pctx", bufs=2)
                        nc.tensor.matmul(pc2[:], hdsb[b][:, hs], al2t[:],
                                         start=True, stop=True)
                        dif = tl(scr, [128, TD], F32, "dif", bufs=2)
                        nc.vector.tensor_tensor(out=dif[:],
                                                in0=st[h][:, c0:c0+TD],
                                                in1=pcd[:], op=ALU.subtract)
                        nc.vector.tensor_tensor(out=dif[:], in0=dif[:],
                                                in1=betabd[:], op=ALU.mult)
                        nc.vector.tensor_tensor(out=dif[:], in0=dif[:],
                                                in1=pcd[:], op=ALU.add)
                        nc.vector.tensor_tensor(out=atout[h][:, c0:c0+TD],
                                                in0=dif[:],
                                                in1=ht[h][:, c0:c0+TD],
                                                op=ALU.add)
                        dft = tl(scr, [128, TT], F32, "dft", bufs=2)
                        nc.vector.tensor_tensor(out=dft[:],
                                                in0=st[h][:, c1:c1+TT],
                                                in1=pct[:], op=ALU.subtract)
                        nc.vector.tensor_tensor(out=dft[:], in0=dft[:],
                                                in1=betabt[:], op=ALU.mult)
                        nc.vector.tensor_tensor(out=dft[:], in0=dft[:],
                                                in1=pct[:], op=ALU.add)
                        nc.vector.tensor_tensor(out=dft[:], in0=dft[:],
                                                in1=pc2[:], op=ALU.add)
                        nc.vector.tensor_tensor(out=atout[h][:, c1:c1+TT],
                                                in0=dft[:],
                                                in1=ht[h][:, c1:c1+TT],
                                                op=ALU.add)

                # write local A^T to bounce + debug
                at_wr = []
                for h in range(NH):
                    d = dma(out=at_bounce[h*128:(h+1)*128, :], in_=atout[h][:])
                    at_wr.append(d)
                    dma(out=ATDBG[h*128:(h+1)*128, :], in_=atout[h][:])

                # ============ Phase B: AllGather ============
                cc = nc.gpsimd.collective_compute(
                    "AllGather", mybir.AluOpType.bypass,
                    replica_groups=[list(range(NC_))],
                    ins=[at_bounce[:]], outs=[ag_out[:]],
                )
                for d in at_wr:
                    add_dep_helper(cc.ins, d.ins, reason="ag after at write")

            # ============ Phase C: classifier ============
            with tc.tile_pool(name="cls_at", bufs=1) as atp, \
                 tc.tile_pool(name="cls_wt", bufs=1) as wtp, \
                 tc.tile_pool(name="cls_ps", bufs=4, space="PSUM") as psp, \
                 tc.tile_pool(name="cls_ev", bufs=1) as evp:
                at = []
                for h in range(NH):
                    t = tl(atp, [128, TOK], BF, f"at{h}")
                    for r in range(NC_):
                        d = dma(out=t[:, r*TPC:(r+1)*TPC],
                                in_=ag_out[r, h*128:(h+1)*128, :])
                        add_dep_helper(d.ins, cc.ins, reason="at read after ag")
                    at.append(t)

                vstart = [0]
                for w in VCHUNKS:
                    vstart.append(vstart[-1] + w)
                for vi, vw in enumerate(VCHUNKS):
                    v0 = vstart[vi]
                    wt = []
                    for h in range(NH):
                        t = tl(wtp, [128, 512], BF, f"wt{h}", bufs=2)
                        dma(out=t[:, :vw], in_=WTM[h*128:(h+1)*128, v0:v0+vw])
                        wt.append(t)
                    bias_c = tl(evp, [128, 512], F32, "biasc", bufs=2)
                    dma(out=bias_c[:, :vw],
                        in_=BI[v0:v0+vw].partition_broadcast(128))
                    for m in range(NM):
                        ps = tl(psp, [128, 512], F32, "ps", bufs=4)
                        for h in range(NH):
                            nc.tensor.matmul(ps[:, :vw],
                                             at[h][:, m*128:(m+1)*128],
                                             wt[h][:, :vw],
                                             start=(h == 0), stop=(h == NH - 1))
                        ev = tl(evp, [128, 512], F32, "ev", bufs=4)
                        nc.vector.tensor_tensor(out=ev[:, :vw], in0=ps[:, :vw],
                                                in1=bias_c[:, :vw], op=ALU.add)
                        dma(out=OUT[m*128:(m+1)*128, v0:v0+vw], in_=ev[:, :vw])

    import concourse.mybir as mybir2
    _split_oversized_waits(nc, mybir2, maxw=1)
    _BUILT = nc
    return nc


def _prep_inputs(i, x_des, x_tit, hiddens_des, hiddens_title, cells_des,
                 cells_title, V, Wx, Whs, Wv_d, Wg_d, Ws_d, Wh_d, Wv_t, Wg_t,
                 Wh_t, Wmlp, bmlp):
    """Build the in_map for core i (batches 2i, 2i+1)."""
    b0, b1 = 2*i, 2*i+1

    def cat_t(a_des, a_tit):
        # -> [H, 320]: [des b0 | tit b0 | des b1 | tit b1]
        return np.concatenate([a_des[b0].T, a_tit[b0].T,
                               a_des[b1].T, a_tit[b1].T], axis=1)

    def shift(h):
        z = np.zeros((1, h.shape[1]), h.dtype)
        return np.concatenate([z, h[:-1]], axis=0)

    hp_des = np.stack([shift(hiddens_des[b0]), shift(hiddens_des[b1])])
    hp_tit = np.stack([shift(hiddens_title[b0]), shift(hiddens_title[b1])])

    XTl = cat_t(x_des, x_tit).astype(bf16)
    HPT = np.concatenate([hp_des[0].T, hp_tit[0].T, hp_des[1].T, hp_tit[1].T],
                         axis=1).astype(bf16)
    HTl = cat_t(hiddens_des, hiddens_title).astype(bf16)
    CTl = cat_t(cells_des, cells_title).astype(np.float32)
    VNl = V[[b0, b1]].astype(bf16)
    VTl = np.ascontiguousarray(V[[b0, b1]].transpose(0, 2, 1)).astype(bf16)
    HDN = hiddens_des[[b0, b1]].astype(bf16)

    wpad = np.zeros((VPAD, H), np.float32)
    wpad[:VOC] = Wmlp
    bpad = np.zeros(VPAD, np.float32)
    bpad[:VOC] = bmlp
    WTMl = np.ascontiguousarray(wpad[i*VPC:(i+1)*VPC].T).astype(bf16)
    BIl = bpad[i*VPC:(i+1)*VPC].copy()

    return dict(
        XT=XTl, HPT=HPT, HT=HTl, CT=CTl, VN=VNl, VT=VTl, HDN=HDN,
        WXT=np.ascontiguousarray(Wx.T).astype(bf16),
        WHST=np.ascontiguousarray(Whs.T).astype(bf16),
        WVDT=np.ascontiguousarray(Wv_d.T).astype(bf16),
        WGDT=np.ascontiguousarray(Wg_d.T).astype(bf16),
        WSDT=np.ascontiguousarray(Ws_d.T).astype(bf16),
        WVTT=np.ascontiguousarray(Wv_t.T).astype(bf16),
        WGTT=np.ascontiguousarray(Wg_t.T).astype(bf16),
        WHD49=np.tile(np.asarray(Wh_d).ravel(), K).astype(np.float32),
        WHD32=np.tile(np.asarray(Wh_d).ravel(), TT).astype(np.float32),
        WHT32=np.tile(np.asarray(Wh_t).ravel(), TT).astype(np.float32),
        WTM=WTMl, BI=BIl,
    )


def kernel(_trace=False, _tmpdir=None, **inputs):
    from concourse.bass_utils import run_bass_kernel_spmd
    if _trace:
        _install_axon_trace_hook()
    nc = _build()
    inputs = {k: np.asarray(v) for k, v in inputs.items()}
    in_maps = [_prep_inputs(i, **inputs) for i in range(NC_)]
    res = run_bass_kernel_spmd(nc, in_maps, core_ids=list(range(NC_)),
                               trace=_trace, tmpdir=_tmpdir)
    kernel.last_result = res
    full = np.concatenate([res.results[i]["OUT"] for i in range(NC_)],
                          axis=1)[:, :VOC]
    view = full.reshape(NC_, BPC, TPB, VOC)
    scores_des = np.ascontiguousarray(view[:, :, :TD]).reshape(B, TD, VOC)
    scores_tit = np.ascontiguousarray(view[:, :, TD:]).reshape(B, TT, VOC)
    return (scores_des.astype(np.float32), scores_tit.astype(np.float32))


# revision 6
# speedup vs baseline: 1.0318x; 1.0318x over previous
"""Trainium2 Bass kernel for nn_AdaptiveBlock (8 NeuronCores, SPMD).

Sharding: data-parallel over batch (2 batches/core) for the attention
front-end; split AllGather of the blended activations A^T (one per batch
half, so the classifier on half 0 overlaps the attention of half 1);
vocab-column-parallel final classifier (6283 vocab rows/core of the
padded 50264).
"""

import sys
import types

import numpy as np

sys.path.insert(0, "/opt/trn_rl_repo")

import ml_dtypes  # noqa: E402

bf16 = ml_dtypes.bfloat16

# ---- problem constants (hardcoded per task rules) ----
B, TD, TT, H, E2, K, VOC = 16, 128, 32, 1024, 1024, 49, 50257
NC_ = 8                   # cores
BPC = B // NC_            # batches per core = 2
TPB = TD + TT             # tokens per batch = 160
TOK = B * TPB             # total tokens = 2560
THALF = NC_ * TPB         # tokens per half (one batch per core) = 1280
NMH = THALF // 128        # token chunks per half = 10
VPAD = 50264              # padded vocab (8 * 6283)
VPC = VPAD // NC_         # vocab per core = 6283
NH = H // 128             # h chunks = 8
VCHUNKS = [512] * 12 + [139]   # 12*512 + 139 = 6283
KJ = K * K                # 2401
TJ = TT * K               # 1568


def _install_axon_trace_hook():
    if "antenv.axon_hooks" in sys.modules:
        return
    try:
        import antenv
        mod = types.ModuleType("antenv.axon_hooks")
        state = {"hook": None}
        mod.set_axon_ntff_profile_hook = lambda h: state.__setitem__("hook", h)
        mod.get_axon_ntff_profile_hook = lambda: state["hook"]
        sys.modules["antenv.axon_hooks"] = mod
        antenv.axon_hooks = mod
        if "/root/.axon_site" not in sys.path:
            sys.path.insert(0, "/root/.axon_site")
        from trn_agent_boot.trn_boot import _ntff_profile_via_ctypes
        mod.set_axon_ntff_profile_hook(
            _ntff_profile_via_ctypes("/opt/axon/libaxon_pjrt.so"))
    except Exception:
        pass


def _split_oversized_waits(nc, mybir, maxw=1):
    """This walrus build allows one sync wait per instruction; hoist extras
    onto preceding same-engine NOPs (engines execute their stream in order)."""
    n_new = 0
    for f in nc.m.functions:
        for bb in f.blocks:
            new_insts = []
            for inst in bb.instructions:
                si = inst.sync_info
                if si is not None and len(si.on_wait) > maxw:
                    waits = list(si.on_wait)
                    extra, keep = waits[:-maxw], waits[-maxw:]
                    while extra:
                        chunk, extra = extra[:maxw], extra[maxw:]
                        nop = mybir.InstNoOp(
                            name=f"{inst.name}-wsplit-{n_new}",
                            engine=inst.engine,
                            sync_info=mybir.SyncInfo(on_wait=chunk, on_update=[]),
                        )
                        n_new += 1
                        nc.register_instruction(nop)
                        new_insts.append(nop)
                    si.on_wait = keep
                new_insts.append(inst)
            bb.instructions = new_insts
    return n_new


_BUILT = None


def _build():
    """Build the Bass graph (one SPMD program, same on all 8 cores)."""
    global _BUILT
    if _BUILT is not None:
        return _BUILT

    import concourse.bass as bass
    import concourse.mybir as mybir
    from concourse.tile import TileContext
    from concourse.tile_rust import add_dep_helper
    from concourse.masks import make_identity

    F32 = mybir.dt.float32
    BF = mybir.dt.bfloat16
    AF = mybir.ActivationFunctionType
    ALU = mybir.AluOpType
    AX = mybir.AxisListType

    nc = bass.Bass()
    P = nc.declare_dram_parameter

    # -------- parameters (per-core shards, host-prepared) --------
    # token axis per core: [b0: des 128 | tit 32][b1: des 128 | tit 32]
    XT = P("XT", [NH, 128, 2 * TPB], BF, isOutput=False)    # x^T h-chunked
    HPT = P("HPT", [NH, 128, 2 * TPB], BF, isOutput=False)  # h_prev^T
    HT = P("HT", [NH, 128, 2 * TPB], BF, isOutput=False)    # hiddens^T
    CT = P("CT", [NH, 128, 2 * TPB], BF, isOutput=False)    # cells^T
    VN = P("VN", [BPC, K, H], BF, isOutput=False)           # V natural
    VT = P("VT", [BPC, NH, 128, K], BF, isOutput=False)     # V^T
    HDN = P("HDN", [BPC, TD, H], BF, isOutput=False)        # hiddens_des nat
    WXT = P("WXT", [NH, 128, H], BF, isOutput=False)
    WHST = P("WHST", [NH, 128, H], BF, isOutput=False)
    WSM = P("WSM", [5, NH, 128, K], BF, isOutput=False)     # Wv_d,Wg_d,Ws_d,Wv_t,Wg_t (^T)
    WHD49 = P("WHD49", [KJ], BF, isOutput=False)            # tile(Wh_d, 49)
    WHD32 = P("WHD32", [TJ], BF, isOutput=False)            # tile(Wh_d, 32)
    WHT32 = P("WHT32", [TJ], BF, isOutput=False)            # tile(Wh_t, 32)
    WTM = P("WTM", [NH, 128, VPC], BF, isOutput=False)      # Wmlp^T slice
    BI = P("BI", [VPC], F32, isOutput=False)                # bias slice
    OUT = P("OUT", [TOK, VPC], F32, isOutput=True)
    ATDBG = P("ATDBG", [H, 2 * TPB], BF, isOutput=True)     # debug: local A^T

    # -------- internal DRAM --------
    vw_dr = nc.dram_tensor("vw_dr", [BPC, K, K], F32)
    hgt_dr = nc.dram_tensor("hgt_dr", [BPC, TT, K], F32)
    htg_dr = nc.dram_tensor("htg_dr", [BPC, TT, K], F32)
    beta_dr = nc.dram_tensor("beta_dr", [BPC, 2, TD], F32)
    at_bounce = [nc.dram_tensor(f"at_bounce{b}", [H, TPB], BF)
                 for b in range(BPC)]
    ag_out = [nc.dram_tensor(f"ag_out{b}", [NC_, H, TPB], BF,
                             addr_space="Shared") for b in range(BPC)]

    with TileContext(nc) as tc:
        with tc.tile_pool(name="const", bufs=1) as cp, \
             tc.tile_pool(name="sent", bufs=1) as sp, \
             tc.tile_pool(name="attn", bufs=1) as ap_, \
             tc.tile_pool(name="scr", bufs=1) as scr, \
             tc.tile_pool(name="aps", bufs=1, space="PSUM") as aps, \
             tc.tile_pool(name="cls", bufs=1) as clp, \
             tc.tile_pool(name="cls_ps", bufs=1, space="PSUM") as psp:

            dma = nc.sync.dma_start
            gdma = nc.gpsimd.dma_start

            def tl(pool, shape, dt, tag, bufs=1):
                return pool.tile(shape, dt, tag=tag, name=tag, bufs=bufs)

            # ---- constants (batched single DMAs) ----
            ident = tl(cp, [128, 128], F32, "ident")
            make_identity(nc, ident[:])
            whd49 = tl(cp, [128, KJ], BF, "whd49")
            dma(out=whd49[:], in_=WHD49[:].partition_broadcast(128))
            whd32 = tl(cp, [K, TJ], BF, "whd32")
            dma(out=whd32[:], in_=WHD32[:].partition_broadcast(K))
            wht32 = tl(cp, [128, TJ], BF, "wht32")
            dma(out=wht32[:], in_=WHT32[:].partition_broadcast(128))

            def wide(pool, src3, n, w, tag, dt=BF):
                """src3 [n, 128, w] -> one tile [128, n*w], chunk e at cols
                [e*w, (e+1)*w)."""
                t = tl(pool, [128, n * w], dt, tag)
                dma(out=t[:].rearrange("p (n w) -> p n w", n=n),
                    in_=src3[:].transpose([1, 0, 2]))
                return t

            wsm = wide(cp, WSM.rearrange("f n p k -> (f n) p k"), 5 * NH, K,
                       "wsm")            # [128, 5*8*49]

            def wsm_c(f, e):
                return wsm[:, (f * NH + e) * K:(f * NH + e + 1) * K]

            ht_all = wide(cp, HT, NH, 2 * TPB, "ht_all")
            xt_all = wide(sp, XT, NH, 2 * TPB, "xt_all")
            hpt_all = wide(sp, HPT, NH, 2 * TPB, "hpt_all")
            ct_all = wide(sp, CT, NH, 2 * TPB, "ct_all")
            wxt_all = wide(sp, WXT, NH, H, "wxt_all")
            whst_all = wide(sp, WHST, NH, H, "whst_all")
            vsb, vt_all, hdsb = [], [], []
            for b in range(BPC):
                v = tl(cp, [K, H], BF, f"vn{b}")
                dma(out=v[:], in_=VN[b])
                vsb.append(v)
                vt_all.append(wide(cp, VT[b], NH, K, f"vt{b}"))
                hd = tl(cp, [TD, H], BF, f"hd{b}")
                dma(out=hd[:], in_=HDN[b])
                hdsb.append(hd)

            def htc(e, lo, n):
                return ht_all[:, e * 2 * TPB + lo: e * 2 * TPB + lo + n]

            # classifier tiles (shared across both halves)
            at = [tl(clp, [128, THALF], BF, f"at{h}", bufs=2)
                  for h in range(NH)]
            vstart = [0]
            for w_ in VCHUNKS:
                vstart.append(vstart[-1] + w_)

            def load_wt(v0, vw_):
                t = tl(clp, [128, NH * 512], BF, "wtall", bufs=2)
                dma(out=t[:].rearrange("p (n w) -> p n w", n=NH)[:, :, :vw_],
                    in_=WTM[:, :, v0:v0 + vw_].transpose([1, 0, 2]))
                return t

            # prefetch first two weight chunks during attention
            wt_pre = [load_wt(vstart[0], VCHUNKS[0]),
                      load_wt(vstart[1], VCHUNKS[1])]

            def mm_acc(ps, lhs_list, rhs_list):
                n = len(lhs_list)
                for e in range(n):
                    nc.tensor.matmul(ps, lhs_list[e], rhs_list[e],
                                     start=(e == 0), stop=(e == n - 1))

            def transpose(src_ap, p, f):
                pt = tl(aps, [f, p], F32, "ptr")
                nc.tensor.transpose(pt[:], src_ap, ident[:p, :p])
                out = tl(scr, [f, p], F32, "str", bufs=2)
                nc.vector.tensor_copy(out[:], pt[:])
                return out

            # ================= attention for one batch =================
            def attention(b):
                c0 = b * TPB           # des col start (within 320-token axis)
                c1 = c0 + TD           # tit col start

                # --- independent small matmuls + roundtrips (no sentinel dep)
                p_vw = tl(aps, [K, K], F32, "psm", bufs=2)
                mm_acc(p_vw[:], [vt_all[b][:, e*K:(e+1)*K] for e in range(NH)],
                       [wsm_c(0, e) for e in range(NH)])
                vws = tl(ap_, [K, K], F32, "vws")
                nc.vector.tensor_copy(vws[:], p_vw[:])
                d1 = gdma(out=vw_dr[b], in_=vws[:])
                vwb = tl(ap_, [128, KJ], F32, "vwb")
                d2 = gdma(out=vwb[:], in_=vw_dr[b].rearrange("k j -> (k j)")
                          .partition_broadcast(128))
                add_dep_helper(d2.ins, d1.ins, reason="vw roundtrip")

                p_hgd = tl(aps, [TD, K], F32, "psm", bufs=2)
                mm_acc(p_hgd[:], [htc(e, c0, TD) for e in range(NH)],
                       [wsm_c(1, e) for e in range(NH)])
                hgd = tl(ap_, [TD, K], F32, "hgd")
                nc.vector.tensor_copy(hgd[:], p_hgd[:])

                p_hgt = tl(aps, [TT, K], F32, "psm", bufs=2)
                mm_acc(p_hgt[:], [htc(e, c1, TT) for e in range(NH)],
                       [wsm_c(1, e) for e in range(NH)])
                hgt = tl(ap_, [TT, K], F32, "hgt")
                nc.vector.tensor_copy(hgt[:], p_hgt[:])
                d3 = gdma(out=hgt_dr[b], in_=hgt[:])
                hgtb = tl(ap_, [K, TJ], F32, "hgtb")
                d4 = gdma(out=hgtb[:], in_=hgt_dr[b].rearrange("t j -> (t j)")
                          .partition_broadcast(K))
                add_dep_helper(d4.ins, d3.ins, reason="hgt roundtrip")

                p_htg = tl(aps, [TT, K], F32, "psm", bufs=2)
                mm_acc(p_htg[:], [htc(e, c1, TT) for e in range(NH)],
                       [wsm_c(4, e) for e in range(NH)])
                htgs = tl(ap_, [TT, K], F32, "htgs")
                nc.vector.tensor_copy(htgs[:], p_htg[:])
                d5 = gdma(out=htg_dr[b], in_=htgs[:])
                htgb = tl(ap_, [128, TJ], F32, "htgb")
                d6 = gdma(out=htgb[:], in_=htg_dr[b].rearrange("t j -> (t j)")
                          .partition_broadcast(128))
                add_dep_helper(d6.ins, d5.ins, reason="htg roundtrip")

                p_hdw = tl(aps, [TD, K], F32, "psm", bufs=2)
                mm_acc(p_hdw[:], [htc(e, c0, TD) for e in range(NH)],
                       [wsm_c(3, e) for e in range(NH)])
                hdw = tl(ap_, [TD, K], F32, "hdw")
                nc.vector.tensor_copy(hdw[:], p_hdw[:])

                # --- sentinel for this batch (N=160) ---
                st = []
                for o in range(NH):
                    pg = tl(aps, [128, TPB], F32, "pgate", bufs=1)
                    for e in range(NH):
                        nc.tensor.matmul(
                            pg[:], wxt_all[:, e*H + o*128:e*H + (o+1)*128],
                            xt_all[:, e*2*TPB + c0:e*2*TPB + c0 + TPB],
                            start=(e == 0), stop=False)
                    for e in range(NH):
                        nc.tensor.matmul(
                            pg[:], whst_all[:, e*H + o*128:e*H + (o+1)*128],
                            hpt_all[:, e*2*TPB + c0:e*2*TPB + c0 + TPB],
                            start=False, stop=(e == NH - 1))
                    sig = tl(scr, [128, TPB], F32, "sig", bufs=2)
                    nc.scalar.activation(sig[:], pg[:], AF.Sigmoid)
                    tcl = tl(scr, [128, TPB], F32, "tcl", bufs=2)
                    nc.scalar.activation(
                        tcl[:], ct_all[:, o*2*TPB + c0:o*2*TPB + c0 + TPB],
                        AF.Tanh)
                    s_o = tl(ap_, [128, TPB], BF, f"st{o}")
                    nc.vector.tensor_tensor(out=s_o[:], in0=sig[:], in1=tcl[:],
                                            op=ALU.mult)
                    st.append(s_o)

                # --- sentinel scores input (needs st) ---
                p_sgd = tl(aps, [TD, K], F32, "psm", bufs=2)
                mm_acc(p_sgd[:], [st[e][:, :TD] for e in range(NH)],
                       [wsm_c(2, e) for e in range(NH)])
                sgd = tl(ap_, [TD, K], F32, "sgd")
                nc.vector.tensor_copy(sgd[:], p_sgd[:])
                p_sgt = tl(aps, [TT, K], F32, "psm", bufs=2)
                mm_acc(p_sgt[:], [st[e][:, TD:TPB] for e in range(NH)],
                       [wsm_c(2, e) for e in range(NH)])
                sgt = tl(ap_, [TT, K], F32, "sgt")
                nc.vector.tensor_copy(sgt[:], p_sgt[:])

                # --- z des: [TD, K] over V ---
                cv = tl(scr, [TD, KJ], F32, "cv")
                nc.vector.tensor_tensor(
                    out=cv[:].rearrange("p (k j) -> p k j", k=K),
                    in0=vwb[:TD, :].rearrange("p (k j) -> p k j", k=K),
                    in1=hgd[:].unsqueeze(1).broadcast_to([TD, K, K]),
                    op=ALU.add)
                nc.scalar.activation(cv[:], cv[:], AF.Tanh)
                nc.vector.tensor_tensor(out=cv[:], in0=cv[:],
                                        in1=whd49[:TD, :], op=ALU.mult)
                z_des = tl(ap_, [TD, K], F32, "z_des")
                nc.vector.tensor_reduce(
                    out=z_des[:], in_=cv[:].rearrange("p (k j) -> p k j", k=K),
                    op=ALU.add, axis=AX.X)

                # --- z tit over V: k-partition layout [K, TT*K] ---
                cvt = tl(scr, [K, TJ], F32, "cvt")
                nc.vector.tensor_tensor(
                    out=cvt[:].rearrange("p (t j) -> p t j", t=TT),
                    in0=vws[:].unsqueeze(1).broadcast_to([K, TT, K]),
                    in1=hgtb[:].rearrange("p (t j) -> p t j", t=TT),
                    op=ALU.add)
                nc.scalar.activation(cvt[:], cvt[:], AF.Tanh)
                nc.vector.tensor_tensor(out=cvt[:], in0=cvt[:],
                                        in1=whd32[:, :], op=ALU.mult)
                ztt = tl(scr, [K, TT], F32, "ztt")
                nc.vector.tensor_reduce(
                    out=ztt[:], in_=cvt[:].rearrange("p (t j) -> p t j", t=TT),
                    op=ALU.add, axis=AX.X)
                z_tit = transpose(ztt[:], K, TT)             # [TT, K]

                # --- z2 tit over hd: u-partition layout [TD, TT*K] ---
                cv2 = tl(scr, [TD, TJ], F32, "cv2")
                nc.vector.tensor_tensor(
                    out=cv2[:].rearrange("p (t j) -> p t j", t=TT),
                    in0=hdw[:].unsqueeze(1).broadcast_to([TD, TT, K]),
                    in1=htgb[:TD, :].rearrange("p (t j) -> p t j", t=TT),
                    op=ALU.add)
                nc.scalar.activation(cv2[:], cv2[:], AF.Tanh)
                nc.vector.tensor_tensor(out=cv2[:], in0=cv2[:],
                                        in1=wht32[:TD, :], op=ALU.mult)
                z2t = tl(scr, [TD, TT], F32, "z2t")
                nc.vector.tensor_reduce(
                    out=z2t[:], in_=cv2[:].rearrange("p (t j) -> p t j", t=TT),
                    op=ALU.add, axis=AX.X)
                z2 = transpose(z2t[:], TD, TT)               # [TT, TD]

                # --- zs (sentinel scores) ---
                def zs_path(sg_sb, hg_sb, t, tag):
                    cvs = tl(scr, [t, K], F32, f"cvs{tag}")
                    nc.vector.tensor_tensor(out=cvs[:], in0=sg_sb[:],
                                            in1=hg_sb[:], op=ALU.add)
                    nc.scalar.activation(cvs[:], cvs[:], AF.Tanh)
                    nc.vector.tensor_tensor(out=cvs[:], in0=cvs[:],
                                            in1=whd49[:t, :K], op=ALU.mult)
                    zs = tl(scr, [t, 1], F32, f"zs{tag}")
                    nc.vector.tensor_reduce(out=zs[:], in_=cvs[:],
                                            op=ALU.add, axis=AX.X)
                    return zs

                zs_des = zs_path(sgd, hgd, TD, "d")
                zs_tit = zs_path(sgt, hgt, TT, "t")

                # --- softmax + beta, per branch ---
                def soft_beta(z, zs, t, beta_slot, tag):
                    m1 = tl(scr, [t, 1], F32, f"m1{tag}")
                    nc.vector.tensor_reduce(out=m1[:], in_=z[:], op=ALU.max,
                                            axis=AX.X, negate=True)
                    e = tl(scr, [t, K], F32, f"esm{tag}")
                    s1 = tl(scr, [t, 1], F32, f"s1{tag}")
                    nc.scalar.activation(e[:], z[:], AF.Exp, bias=m1[:],
                                         accum_out=s1[:])
                    es = tl(scr, [t, 1], F32, f"es{tag}")
                    nc.scalar.activation(es[:], zs[:], AF.Exp, bias=m1[:])
                    den = tl(scr, [t, 1], F32, f"den{tag}")
                    nc.vector.tensor_tensor(out=den[:], in0=s1[:], in1=es[:],
                                            op=ALU.add)
                    rden = tl(scr, [t, 1], F32, f"rden{tag}")
                    nc.vector.reciprocal(rden[:], den[:])
                    beta = tl(scr, [t, 1], F32, f"beta{tag}")
                    nc.vector.tensor_tensor(out=beta[:], in0=es[:],
                                            in1=rden[:], op=ALU.mult)
                    rs1 = tl(scr, [t, 1], F32, f"rs1{tag}")
                    nc.vector.reciprocal(rs1[:], s1[:])
                    alpha = tl(scr, [t, K], F32, f"alpha{tag}")
                    nc.vector.tensor_scalar(out=alpha[:], in0=e[:],
                                            scalar1=rs1[:], scalar2=None,
                                            op0=ALU.mult)
                    alt_f = transpose(alpha[:], t, K)
                    alt = tl(scr, [K, t], BF, f"altb{tag}", bufs=2)
                    nc.vector.tensor_copy(alt[:], alt_f[:])
                    da = gdma(out=beta_dr[b, beta_slot, :t], in_=beta[:])
                    bb_ = tl(scr, [128, t], F32, f"betab{tag}", bufs=2)
                    db = gdma(out=bb_[:], in_=beta_dr[b, beta_slot, :t]
                              .partition_broadcast(128))
                    add_dep_helper(db.ins, da.ins, reason="beta roundtrip")
                    return alt, bb_

                altd, betabd = soft_beta(z_des, zs_des, TD, 0, "d")
                altt, betabt = soft_beta(z_tit, zs_tit, TT, 1, "t")

                # alpha2 = softmax(z2) [TT, TD] -> alpha2T [TD, TT] bf16
                m2 = tl(scr, [TT, 1], F32, "m2")
                nc.vector.tensor_reduce(out=m2[:], in_=z2[:], op=ALU.max,
                                        axis=AX.X, negate=True)
                e2_ = tl(scr, [TT, TD], F32, "e2")
                s2 = tl(scr, [TT, 1], F32, "s2")
                nc.scalar.activation(e2_[:], z2[:], AF.Exp, bias=m2[:],
                                     accum_out=s2[:])
                rs2 = tl(scr, [TT, 1], F32, "rs2")
                nc.vector.reciprocal(rs2[:], s2[:])
                al2 = tl(scr, [TT, TD], F32, "al2")
                nc.vector.tensor_scalar(out=al2[:], in0=e2_[:], scalar1=rs2[:],
                                        scalar2=None, op0=ALU.mult)
                al2t_f = transpose(al2[:], TT, TD)
                al2t = tl(scr, [TD, TT], BF, "al2tb", bufs=2)
                nc.vector.tensor_copy(al2t[:], al2t_f[:])

                # --- contexts + blends per h-chunk -> atout ---
                atout = [tl(ap_, [128, TPB], BF, f"ato{h}") for h in range(NH)]
                for h in range(NH):
                    hs = slice(h*128, (h+1)*128)
                    pcd = tl(aps, [128, TD], F32, "pctx", bufs=1)
                    nc.tensor.matmul(pcd[:], vsb[b][:, hs], altd[:],
                                     start=True, stop=True)
                    pct = tl(aps, [128, TT], F32, "pctx", bufs=1)
                    nc.tensor.matmul(pct[:], vsb[b][:, hs], altt[:],
                                     start=True, stop=True)
                    pc2 = tl(aps, [128, TT], F32, "pctx", bufs=1)
                    nc.tensor.matmul(pc2[:], hdsb[b][:, hs], al2t[:],
                                     start=True, stop=True)
                    dif = tl(scr, [128, TD], F32, "dif", bufs=2)
                    nc.vector.tensor_tensor(out=dif[:], in0=st[h][:, :TD],
                                            in1=pcd[:], op=ALU.subtract)
                    nc.vector.tensor_tensor(out=dif[:], in0=dif[:],
                                            in1=betabd[:], op=ALU.mult)
                    nc.vector.tensor_tensor(out=dif[:], in0=dif[:],
                                            in1=pcd[:], op=ALU.add)
                    nc.vector.tensor_tensor(out=atout[h][:, :TD], in0=dif[:],
                                            in1=htc(h, c0, TD), op=ALU.add)
                    dft = tl(scr, [128, TT], F32, "dft", bufs=2)
                    nc.vector.tensor_tensor(out=dft[:], in0=st[h][:, TD:TPB],
                                            in1=pct[:], op=ALU.subtract)
                    nc.vector.tensor_tensor(out=dft[:], in0=dft[:],
                                            in1=betabt[:], op=ALU.mult)
                    nc.vector.tensor_tensor(out=dft[:], in0=dft[:],
                                            in1=pct[:], op=ALU.add)
                    nc.vector.tensor_tensor(out=dft[:], in0=dft[:],
                                            in1=pc2[:], op=ALU.add)
                    nc.vector.tensor_tensor(out=atout[h][:, TD:TPB],
                                            in0=dft[:], in1=htc(h, c1, TT),
                                            op=ALU.add)

                # bounce + AllGather for this half
                at_wr = []
                for h in range(NH):
                    d = dma(out=at_bounce[b][h*128:(h+1)*128, :],
                            in_=atout[h][:])
                    at_wr.append(d)
                    dma(out=ATDBG[h*128:(h+1)*128, c0:c0+TPB], in_=atout[h][:])
                cc = nc.gpsimd.collective_compute(
                    "AllGather", mybir.AluOpType.bypass,
                    replica_groups=[list(range(NC_))],
                    ins=[at_bounce[b][:]], outs=[ag_out[b][:]],
                )
                for d in at_wr:
                    add_dep_helper(cc.ins, d.ins, reason="ag after at write")
                return cc

            # ================= classifier for one half =================
            def classifier(b, cc):
                # gather A^T for this half: one DMA per h-chunk
                for h in range(NH):
                    d = dma(out=at[h][:].rearrange("p (r t) -> p r t", r=NC_),
                            in_=ag_out[b][:, h*128:(h+1)*128, :]
                            .transpose([1, 0, 2]))
                    add_dep_helper(d.ins, cc.ins, reason="at read after ag")
                for vi, vw_ in enumerate(VCHUNKS):
                    v0 = vstart[vi]
                    if b == 0 and vi < 2:
                        wt = wt_pre[vi]
                    else:
                        wt = load_wt(v0, vw_)
                    bias_c = tl(clp, [128, 512], F32, "biasc", bufs=2)
                    gdma(out=bias_c[:, :vw_],
                         in_=BI[v0:v0+vw_].partition_broadcast(128))
                    for m in range(NMH):
                        ps = tl(psp, [128, 512], F32, "ps", bufs=3)
                        for h in range(NH):
                            nc.tensor.matmul(ps[:, :vw_],
                                             at[h][:, m*128:(m+1)*128],
                                             wt[:, h*512:h*512+vw_],
                                             start=(h == 0),
                                             stop=(h == NH - 1))
                        ev = tl(clp, [128, 512], F32, "ev", bufs=3)
                        nc.vector.tensor_tensor(out=ev[:, :vw_],
                                                in0=ps[:, :vw_],
                                                in1=bias_c[:, :vw_],
                                                op=ALU.add)
                        dma(out=OUT[b*THALF + m*128:b*THALF + (m+1)*128,
                                    v0:v0+vw_],
                            in_=ev[:, :vw_])

            cc0 = attention(0)
            cc1 = attention(1)
            classifier(0, cc0)
            classifier(1, cc1)

    import concourse.mybir as mybir2
    _split_oversized_waits(nc, mybir2, maxw=1)
    _BUILT = nc
    return nc


def _prep_inputs(i, x_des, x_tit, hiddens_des, hiddens_title, cells_des,
                 cells_title, V, Wx, Whs, Wv_d, Wg_d, Ws_d, Wh_d, Wv_t, Wg_t,
                 Wh_t, Wmlp, bmlp):
    """Build the in_map for core i (batches 2i, 2i+1)."""
    b0, b1 = 2*i, 2*i+1

    def hchunk(a):
        # [H, T] -> [NH, 128, T]
        return np.ascontiguousarray(a.reshape(NH, 128, a.shape[1]))

    def cat_t(a_des, a_tit):
        return np.concatenate([a_des[b0].T, a_tit[b0].T,
                               a_des[b1].T, a_tit[b1].T], axis=1)

    def shift(h):
        z = np.zeros((1, h.shape[1]), h.dtype)
        return np.concatenate([z, h[:-1]], axis=0)

    hp = np.concatenate([shift(hiddens_des[b0]).T, shift(hiddens_title[b0]).T,
                         shift(hiddens_des[b1]).T, shift(hiddens_title[b1]).T],
                        axis=1)

    wsm = np.stack([
        hchunk(np.ascontiguousarray(w.T)) for w in
        (Wv_d, Wg_d, Ws_d, Wv_t, Wg_t)
    ])  # [5, NH, 128, K]

    wpad = np.zeros((VPAD, H), np.float32)
    wpad[:VOC] = Wmlp
    bpad = np.zeros(VPAD, np.float32)
    bpad[:VOC] = bmlp
    WTMl = hchunk(np.ascontiguousarray(
        wpad[i*VPC:(i+1)*VPC].T)).astype(bf16)
    BIl = bpad[i*VPC:(i+1)*VPC].copy()

    VTl = np.stack([hchunk(np.ascontiguousarray(V[b].T)) for b in (b0, b1)])

    return dict(
        XT=hchunk(cat_t(x_des, x_tit)).astype(bf16),
        HPT=hchunk(hp).astype(bf16),
        HT=hchunk(cat_t(hiddens_des, hiddens_title)).astype(bf16),
        CT=hchunk(cat_t(cells_des, cells_title)).astype(bf16),
        VN=V[[b0, b1]].astype(bf16),
        VT=VTl.astype(bf16),
        HDN=hiddens_des[[b0, b1]].astype(bf16),
        WXT=hchunk(np.ascontiguousarray(Wx.T)).astype(bf16),
        WHST=hchunk(np.ascontiguousarray(Whs.T)).astype(bf16),
        WSM=wsm.astype(bf16),
        WHD49=np.tile(np.asarray(Wh_d).ravel(), K).astype(bf16),
        WHD32=np.tile(np.asarray(Wh_d).ravel(), TT).astype(bf16),
        WHT32=np.tile(np.asarray(Wh_t).ravel(), TT).astype(bf16),
        WTM=WTMl, BI=BIl,
    )


def kernel(_trace=False, _tmpdir=None, **inputs):
    from concourse.bass_utils import run_bass_kernel_spmd
    if _trace:
        _install_axon_trace_hook()
    nc = _build()
    inputs = {k: np.asarray(v) for k, v in inputs.items()}
    in_maps = [_prep_inputs(i, **inputs) for i in range(NC_)]
    res = run_bass_kernel_spmd(nc, in_maps, core_ids=list(range(NC_)),
                               trace=_trace, tmpdir=_tmpdir)
    kernel.last_result = res
    full = np.concatenate([res.results[i]["OUT"] for i in range(NC_)],
                          axis=1)[:, :VOC]
    # rows: [half 0: core0 b0 | core1 b2 | ...][half 1: core0 b1 | ...]
    view = full.reshape(BPC, NC_, TPB, VOC)
    scores_des = np.empty((B, TD, VOC), np.float32)
    scores_tit = np.empty((B, TT, VOC), np.float32)
    for half in range(BPC):
        for r in range(NC_):
            scores_des[2*r + half] = view[half, r, :TD]
            scores_tit[2*r + half] = view[half, r, TD:]
    return (scores_des, scores_tit)
